# revision 10
# baseline (speedup 1.0000x reference)
"""Trainium2 Bass kernel for GCNBlock (spectral-norm linear + GCN aggregation +
InstanceNorm + LeakyReLU) distributed across 8 NeuronCores.

Strategy (per the dst-sharding hint):
  - out = A @ (x @ WnT) = (A @ x) @ WnT: aggregate raw x rows first, then
    apply the 128x128 weight per dst tile.
  - dst nodes sharded across 8 cores (6272 nodes = 49 tiles of 128 per core).
    Real edges partitioned by dst, grouped per (tile, src-half), DEDUPED per
    group (duplicate srcs share a gather slot; S rows become multi-hot),
    padded to 128-slot blocks (src halves because dma_gather indices are
    int16).  Self loops are folded in via a resident x tile + diagonal S.
  - The scatter matrices S[slot, dst] = coef * onehot are precomputed on the
    host and streamed in bf16 via HWDGE.  The serial bottleneck is GPSIMD's
    SWDGE descriptor emission for the gathers (~8 ns/row, measured); all
    other engines are kept quiet to avoid inflating it -- in particular the
    out-stage runs on the Scalar engine (one fused Lrelu activation) because
    DVE shares its SBUF port with GPSIMD.
  - Per block: bulk-gather 128 src rows of x (bf16) via dma_gather, then
    aggT[cin, dst] += Xsrc.T @ S on the PE in PSUM.
  - Per dst tile: out = aggT.T @ WnT + b (PE), InstanceNorm via
    bn_stats/bn_aggr, then one ACT Lrelu(scale*x+bias, alpha=0.2), DMA out.
"""

import numpy as np
import ml_dtypes
from contextlib import ExitStack

import concourse.tile as tile
from concourse import bacc, mybir
from concourse.bass_utils import run_bass_kernel_spmd

# Problem constants (hardcoded per spec)
N, E, C = 50000, 800000, 128
P = 128
NCORES = 8
TPC = 49                # dst tiles per core
NPC = TPC * P           # 6272 dst nodes per core
NPAD = NCORES * NPC     # 50176 padded node count
HALF = 32768            # int16 index split point
# chunk spans of dst tiles; last chunk is a single tile to shrink the tail
SPANS = [(i * 4, i * 4 + 4) for i in range(12)] + [(48, 49)]
NCHUNKS = len(SPANS)
EPS_IN = 1e-5


def _preprocess(x, edge_index, W, b, u):
    """Host-side prep: spectral norm, edge partitioning, S-matrix layout."""
    x = np.asarray(x, dtype=np.float32)
    ei = np.asarray(edge_index)
    W = np.asarray(W, dtype=np.float32)
    b = np.asarray(b, dtype=np.float32)
    u = np.asarray(u, dtype=np.float32)

    # --- spectral norm (one power iteration), matches reference ---
    eps = np.float32(1e-12)
    v = (W.T @ u).astype(np.float32)
    v = v / (np.float32(np.linalg.norm(v)) + eps)
    Wv = (W @ v).astype(np.float32)
    u2 = Wv / (np.float32(np.linalg.norm(Wv)) + eps)
    sigma = np.float32(u2 @ Wv)
    WnT = np.ascontiguousarray((W / sigma).T, dtype=np.float32)  # [cin, cout]

    src = ei[0].astype(np.int64)
    dst = ei[1].astype(np.int64)

    # --- degrees / coefficients (self loops in degree only) ---
    deg = (np.bincount(dst, minlength=N) + 1).astype(np.float32)
    dinv = (1.0 / np.sqrt(deg)).astype(np.float32)
    coef = dinv[src] * dinv[dst]

    core = dst // NPC
    tile_g = (dst % NPC) // P
    dstloc = dst % P
    half = (src >= HALF).astype(np.int64)

    # --- dedup: gather slot key is (core, tile, half, src) ---
    key = ((core * TPC + tile_g) * 2 + half).astype(np.int64)
    NG = NCORES * TPC * 2
    slotkey = key * 65536 + (src - half * HALF)
    uniq, inv = np.unique(slotkey, return_inverse=True)
    u_key = (uniq // 65536).astype(np.int64)
    u_src = (uniq % 65536).astype(np.int64)
    ucounts = np.bincount(u_key, minlength=NG)
    ustarts = np.zeros(NG + 1, dtype=np.int64)
    np.cumsum(ucounts, out=ustarts[1:])
    urank = np.arange(len(uniq), dtype=np.int64) - ustarts[u_key]

    cnt3 = ucounts.reshape(NCORES, TPC, 2)
    nb = np.ceil(cnt3.max(axis=0) / P).astype(np.int64)  # [TPC, 2]

    # gather-block order: chunk-major, then half, then tile within chunk
    blk_off = np.zeros((TPC, 2), dtype=np.int64)
    gather_blk0 = np.zeros((NCHUNKS, 2), dtype=np.int64)
    gather_nblk = np.zeros((NCHUNKS, 2), dtype=np.int64)
    pos = 0
    for c, (t0, t1) in enumerate(SPANS):
        for h in range(2):
            gather_blk0[c, h] = pos
            for t in range(t0, t1):
                blk_off[t, h] = pos
                pos += nb[t, h]
            gather_nblk[c, h] = pos - gather_blk0[c, h]
    totblk = pos

    # S-block order: chunk-major; self blocks of the chunk's tiles first,
    # then the chunk's gather blocks in gather order.
    s_off_self = np.zeros(TPC, dtype=np.int64)
    s_off_gblk = np.zeros(totblk, dtype=np.int64)
    s_chunk0 = np.zeros(NCHUNKS + 1, dtype=np.int64)
    spos = 0
    for c, (t0, t1) in enumerate(SPANS):
        s_chunk0[c] = spos
        for t in range(t0, t1):
            s_off_self[t] = spos
            spos += 1
        for h in range(2):
            for t in range(t0, t1):
                for j in range(int(nb[t, h])):
                    s_off_gblk[blk_off[t, h] + j] = spos
                    spos += 1
    s_chunk0[NCHUNKS] = spos
    tot_sblk = spos

    # per-unique-slot placement
    u_core = u_key // (TPC * 2)
    u_tile = (u_key // 2) % TPC
    u_half = u_key % 2
    u_blk = blk_off[u_tile, u_half] + urank // P
    u_slot = urank % P

    IDXALL = np.zeros((NCORES, totblk * P), dtype=np.int16)
    IDXALL[u_core, u_blk * P + u_slot] = u_src.astype(np.int16)

    # per-edge S entries (scatter-add: multi-hot rows for duplicate srcs)
    SBLK = np.zeros((NCORES, P, tot_sblk, P), dtype=np.float32)
    np.add.at(
        SBLK,
        (u_core[inv], u_slot[inv], s_off_gblk[u_blk[inv]], dstloc),
        coef,
    )

    # self blocks: diagonal coef for the tile's own nodes
    selfcoef = np.zeros(NPAD, dtype=np.float32)
    selfcoef[:N] = dinv[:N] * dinv[:N]
    sc = selfcoef.reshape(NCORES, TPC, P)
    ar = np.arange(P)
    for cc in range(NCORES):
        for t in range(TPC):
            SBLK[cc, ar, s_off_self[t], ar] = sc[cc, t]

    S_D = np.ascontiguousarray(
        SBLK.reshape(NCORES, P, tot_sblk * P)
    ).astype(ml_dtypes.bfloat16)

    # idx SBUF layout: pos k -> [k % 16, k // 16], replicated 8x over partitions
    IDX = np.tile(IDXALL.reshape(NCORES, -1, 16).transpose(0, 2, 1), (1, 8, 1))

    x_pad = np.zeros((NPAD, C), dtype=ml_dtypes.bfloat16)
    x_pad[:N] = x.astype(ml_dtypes.bfloat16)

    # per-core resident copy of the core's own dst rows, [128, TPC*128]
    xs = x_pad.reshape(NCORES, TPC, P, C).transpose(0, 2, 1, 3).reshape(
        NCORES, P, TPC * C
    )
    X_SELF = np.ascontiguousarray(xs)

    meta = dict(
        nb=nb,
        blk_off=blk_off,
        gather_blk0=gather_blk0,
        gather_nblk=gather_nblk,
        totblk=totblk,
        s_off_self=s_off_self,
        s_off_gblk=s_off_gblk,
        s_chunk0=s_chunk0,
        tot_sblk=tot_sblk,
    )
    return x_pad, IDX, S_D, X_SELF, WnT, b.reshape(1, C), meta


def _build(meta):
    """Build the SPMD Bass graph (shared across all 8 cores)."""
    nb = meta["nb"]
    blk_off = meta["blk_off"]
    gather_blk0 = meta["gather_blk0"]
    gather_nblk = meta["gather_nblk"]
    totblk = meta["totblk"]
    s_off_self = meta["s_off_self"]
    s_off_gblk = meta["s_off_gblk"]
    s_chunk0 = meta["s_chunk0"]
    tot_sblk = meta["tot_sblk"]

    nc = bacc.Bacc("TRN2", target_bir_lowering=False, debug=False)

    x_d = nc.dram_tensor("x", [NPAD, C], mybir.dt.bfloat16, kind="ExternalInput")
    idx_d = nc.dram_tensor("idx", [P, totblk * 8], mybir.dt.int16, kind="ExternalInput")
    s_d = nc.dram_tensor("s", [P, tot_sblk * P], mybir.dt.bfloat16, kind="ExternalInput")
    xself_d = nc.dram_tensor("xself", [P, TPC * C], mybir.dt.bfloat16, kind="ExternalInput")
    wnT_d = nc.dram_tensor("wnT", [C, C], mybir.dt.float32, kind="ExternalInput")
    b_d = nc.dram_tensor("b", [1, C], mybir.dt.float32, kind="ExternalInput")
    out_d = nc.dram_tensor("out", [NPC, C], mybir.dt.float32, kind="ExternalOutput")

    nbc_max = int(gather_nblk.sum(axis=1).max())
    nsc_max = int((s_chunk0[1:] - s_chunk0[:-1]).max())

    with tile.TileContext(nc) as tc, ExitStack() as ctx:
        meta_p = ctx.enter_context(tc.tile_pool(name="meta", bufs=1))
        idx_p = ctx.enter_context(tc.tile_pool(name="idx", bufs=3))
        gat_p = ctx.enter_context(tc.tile_pool(name="gat", bufs=4))
        s_p = ctx.enter_context(tc.tile_pool(name="s", bufs=3))
        agg_p = ctx.enter_context(tc.tile_pool(name="agg", bufs=4))
        out_p = ctx.enter_context(tc.tile_pool(name="out", bufs=4))
        small_p = ctx.enter_context(tc.tile_pool(name="small", bufs=8))
        ps_agg = ctx.enter_context(tc.tile_pool(name="ps_agg", bufs=4, space="PSUM"))
        ps_out = ctx.enter_context(tc.tile_pool(name="ps_out", bufs=3, space="PSUM"))

        # warm-up: tiny gather to overlap the GPSIMD ucode IRAM load with the
        # initial meta DMAs (first SWDGE call pays ~6us otherwise)
        warm_idx = meta_p.tile([P, 8], mybir.dt.int16)
        nc.vector.memset(warm_idx[:], 0)
        warm_out = meta_p.tile([P, 1, P], mybir.dt.bfloat16)
        nc.gpsimd.dma_gather(
            out_ap=warm_out[:],
            in_ap=x_d[0:HALF, :],
            idxs_ap=warm_idx[:],
            num_idxs=P,
            num_idxs_reg=P,
            elem_size=C,
            single_packet=False,
        )

        x_lo = x_d[0:HALF, :]
        x_hi = x_d[HALF:NPAD, :]

        nbi_max = int((gather_nblk[:, 0] + gather_nblk[:, 1]).max())

        def issue_gather(ci):
            cblk0 = int(gather_blk0[ci, 0])
            nblk_c = int(gather_nblk[ci, 0] + gather_nblk[ci, 1])
            csblk0 = int(s_chunk0[ci])
            ns_c = int(s_chunk0[ci + 1]) - csblk0

            idx_sb = idx_p.tile([P, nbi_max * 8], mybir.dt.int16, tag="idx")
            nc.sync.dma_start(
                idx_sb[:, : nblk_c * 8], idx_d[:, cblk0 * 8 : (cblk0 + nblk_c) * 8]
            )

            s_sb = s_p.tile([P, nsc_max * P], mybir.dt.bfloat16, tag="s")
            nc.sync.dma_start(
                s_sb[:, : ns_c * P], s_d[:, csblk0 * P : (csblk0 + ns_c) * P]
            )

            gat_sb = gat_p.tile([P, nbc_max, P], mybir.dt.bfloat16, tag="gat")
            for h, src_ap in ((0, x_lo), (1, x_hi)):
                nblk_g = int(gather_nblk[ci, h])
                if nblk_g == 0:
                    continue
                nidx = nblk_g * P
                g0 = int(gather_blk0[ci, h]) - cblk0
                nc.gpsimd.dma_gather(
                    out_ap=gat_sb[:, g0 : g0 + nblk_g, :],
                    in_ap=src_ap,
                    idxs_ap=idx_sb[:, g0 * 8 : g0 * 8 + nidx // 16],
                    num_idxs=nidx,
                    num_idxs_reg=nidx,
                    elem_size=C,
                    single_packet=False,
                )
            return s_sb, gat_sb

        # chunk 0's gather goes first in program order so its DMA-sem lane
        # isn't queued behind the large meta loads
        pending = issue_gather(0)

        xself_sb = meta_p.tile([P, TPC * C], mybir.dt.bfloat16)
        nc.sync.dma_start(xself_sb[:], xself_d[:])
        wnT_sb = meta_p.tile([C, C], mybir.dt.float32)
        nc.sync.dma_start(wnT_sb[:], wnT_d[:])
        b_sb = meta_p.tile([1, C], mybir.dt.float32)
        nc.sync.dma_start(b_sb[:], b_d[:])
        ones_sb = meta_p.tile([1, C], mybir.dt.float32)
        nc.vector.memset(ones_sb[:], 1.0)
        eps_sb = meta_p.tile([P, 1], mybir.dt.float32)
        nc.vector.memset(eps_sb[:], EPS_IN)

        for ci, (t0, t1) in enumerate(SPANS):
            s_sb, gat_sb = pending
            if ci + 1 < NCHUNKS:
                pending = issue_gather(ci + 1)
            cblk0 = int(gather_blk0[ci, 0])
            csblk0 = int(s_chunk0[ci])

            for t in range(t0, t1):
                # (lhsT source, S block index within chunk) accumulation list
                mms = [("self", int(s_off_self[t]) - csblk0)]
                for h in range(2):
                    for j in range(int(nb[t, h])):
                        g = int(blk_off[t, h]) + j
                        mms.append((g - cblk0, int(s_off_gblk[g]) - csblk0))

                pt = ps_agg.tile([P, P], mybir.dt.float32)
                for j, (lsrc, sc_col) in enumerate(mms):
                    if lsrc == "self":
                        lhsT = xself_sb[:, t * C : (t + 1) * C]
                    else:
                        lhsT = gat_sb[:, lsrc, :]
                    nc.tensor.matmul(
                        pt[:],
                        lhsT=lhsT,
                        rhs=s_sb[:, sc_col * P : (sc_col + 1) * P],
                        start=(j == 0),
                        stop=(j == len(mms) - 1),
                    )

                agg_sb = agg_p.tile([P, P], mybir.dt.float32)
                nc.scalar.copy(agg_sb[:], pt[:])

                po = ps_out.tile([P, P], mybir.dt.float32)
                nc.tensor.matmul(po[:], lhsT=agg_sb[:], rhs=wnT_sb[:], start=True, stop=False)
                nc.tensor.matmul(po[:], lhsT=ones_sb[:], rhs=b_sb[:], start=False, stop=True)

                # InstanceNorm + LeakyReLU: stats on DVE (small), the full-tile
                # normalize+activate fused into one ACT Lrelu op.
                stats = small_p.tile([P, 6], mybir.dt.float32)
                nc.vector.bn_stats(out=stats[:], in_=po[:])
                mv = small_p.tile([P, 2], mybir.dt.float32)
                nc.vector.bn_aggr(out=mv[:], in_=stats[:])
                std = small_p.tile([P, 1], mybir.dt.float32)
                nc.scalar.activation(
                    out=std[:], in_=mv[:, 1:2],
                    func=mybir.ActivationFunctionType.Sqrt,
                    bias=eps_sb[:], scale=1.0,
                )
                rstd = small_p.tile([P, 1], mybir.dt.float32)
                nc.vector.reciprocal(out=rstd[:], in_=std[:])
                nbias = small_p.tile([P, 1], mybir.dt.float32)
                nc.vector.tensor_scalar(
                    out=nbias[:], in0=mv[:, 0:1], scalar1=rstd[:], scalar2=-1.0,
                    op0=mybir.AluOpType.mult, op1=mybir.AluOpType.mult,
                )
                y_sb = out_p.tile([P, P], mybir.dt.float32, tag="y")
                nc.scalar.activation(
                    out=y_sb[:], in_=po[:],
                    func=mybir.ActivationFunctionType.Identity,
                    bias=nbias[:], scale=rstd[:],
                )
                final = out_p.tile([P, P], mybir.dt.float32, tag="final")
                nc.vector.scalar_tensor_tensor(
                    out=final[:], in0=y_sb[:], scalar=0.2, in1=y_sb[:],
                    op0=mybir.AluOpType.mult, op1=mybir.AluOpType.max,
                )
                nc.sync.dma_start(out_d[t * P : (t + 1) * P, :], final[:])

    nc.compile()
    return nc


def _make_in_maps(x_pad, IDX, S_D, X_SELF, WnT, bvec):
    return [
        {
            "x": x_pad,
            "idx": np.ascontiguousarray(IDX[i]),
            "s": np.ascontiguousarray(S_D[i]),
            "xself": np.ascontiguousarray(X_SELF[i]),
            "wnT": WnT,
            "b": bvec,
        }
        for i in range(NCORES)
    ]


def kernel(x, edge_index, W, b, u):
    x_pad, IDX, S_D, X_SELF, WnT, bvec, meta = _preprocess(x, edge_index, W, b, u)
    nc = _build(meta)
    in_maps = _make_in_maps(x_pad, IDX, S_D, X_SELF, WnT, bvec)

    # The axon terminal can be transiently unavailable right after a prior
    # process's teardown; retry with backoff.
    import time

    last_err = None
    for attempt in range(6):
        try:
            res = run_bass_kernel_spmd(nc, in_maps, list(range(NCORES)))
            break
        except Exception as e:  # noqa: BLE001
            last_err = e
            time.sleep(45)
    else:
        raise last_err
    shards = [np.asarray(res.results[i]["out"]) for i in range(NCORES)]
    out = np.concatenate(shards, axis=0)[:N]
    return out.astype(np.float32)


# revision 15
# speedup vs baseline: 1.6115x; 1.6115x over previous
"""Trainium2 Bass kernel for GCNBlock (spectral-norm linear + GCN aggregation +
InstanceNorm + LeakyReLU) distributed across 8 NeuronCores.

Strategy (per the dst-sharding hint):
  - out = A @ (x @ WnT) = (A @ x) @ WnT: aggregate raw x rows first, then
    apply the 128x128 weight per dst tile.
  - dst nodes sharded across 8 cores (6272 nodes = 49 tiles of 128 per core).
    Real edges partitioned by dst, grouped per (tile, src-half), DEDUPED per
    group (duplicate srcs share a gather slot; S rows become multi-hot),
    padded to 128-slot blocks (src halves because dma_gather indices are
    int16).  Self loops are folded in via a resident x tile + diagonal S.
  - The scatter matrices S[slot, dst] = coef * onehot are precomputed on the
    host and streamed in bf16 via HWDGE.  The serial bottleneck is GPSIMD's
    SWDGE descriptor emission for the gathers (~8 ns/row, measured); all
    other engines are kept quiet to avoid inflating it -- in particular the
    out-stage runs on the Scalar engine (one fused Lrelu activation) because
    DVE shares its SBUF port with GPSIMD.
  - Per block: bulk-gather 128 src rows of x (bf16) via dma_gather, then
    aggT[cin, dst] += Xsrc.T @ S on the PE in PSUM.
  - Per dst tile: out = aggT.T @ WnT + b (PE), InstanceNorm via
    bn_stats/bn_aggr, then one ACT Lrelu(scale*x+bias, alpha=0.2), DMA out.
"""

import numpy as np
import ml_dtypes
from contextlib import ExitStack

import concourse.tile as tile
from concourse import bacc, mybir
from concourse.bass_utils import run_bass_kernel_spmd

# Problem constants (hardcoded per spec)
N, E, C = 50000, 800000, 128
P = 128
NCORES = 8
TPC = 49                # dst tiles per core
NPC = TPC * P           # 6272 dst nodes per core
NPAD = NCORES * NPC     # 50176 padded node count
HALF = 32768            # int16 index split point
# chunk spans of dst tiles; last chunk is a single tile to shrink the tail
SPANS = [(i * 4, i * 4 + 4) for i in range(12)] + [(48, 49)]
NCHUNKS = len(SPANS)
EPS_IN = 1e-5


def _preprocess(x, edge_index, W, b, u):
    """Host-side prep: spectral norm, edge partitioning, S-matrix layout."""
    x = np.asarray(x, dtype=np.float32)
    ei = np.asarray(edge_index)
    W = np.asarray(W, dtype=np.float32)
    b = np.asarray(b, dtype=np.float32)
    u = np.asarray(u, dtype=np.float32)

    # --- spectral norm (one power iteration), matches reference ---
    eps = np.float32(1e-12)
    v = (W.T @ u).astype(np.float32)
    v = v / (np.float32(np.linalg.norm(v)) + eps)
    Wv = (W @ v).astype(np.float32)
    u2 = Wv / (np.float32(np.linalg.norm(Wv)) + eps)
    sigma = np.float32(u2 @ Wv)
    WnT = np.ascontiguousarray((W / sigma).T, dtype=np.float32)  # [cin, cout]

    src = ei[0].astype(np.int64)
    dst = ei[1].astype(np.int64)

    # --- degrees / coefficients (self loops in degree only) ---
    deg = (np.bincount(dst, minlength=N) + 1).astype(np.float32)
    dinv = (1.0 / np.sqrt(deg)).astype(np.float32)
    coef = dinv[src] * dinv[dst]

    core = dst // NPC
    tile_g = (dst % NPC) // P
    dstloc = dst % P
    half = (src >= HALF).astype(np.int64)

    # --- dedup: gather slot key is (core, tile, half, src) ---
    key = ((core * TPC + tile_g) * 2 + half).astype(np.int64)
    NG = NCORES * TPC * 2
    slotkey = key * 65536 + (src - half * HALF)
    uniq, inv = np.unique(slotkey, return_inverse=True)
    u_key = (uniq // 65536).astype(np.int64)
    u_src = (uniq % 65536).astype(np.int64)
    ucounts = np.bincount(u_key, minlength=NG)
    ustarts = np.zeros(NG + 1, dtype=np.int64)
    np.cumsum(ucounts, out=ustarts[1:])
    urank = np.arange(len(uniq), dtype=np.int64) - ustarts[u_key]

    cnt3 = ucounts.reshape(NCORES, TPC, 2)
    nb = np.ceil(cnt3.max(axis=0) / P).astype(np.int64)  # [TPC, 2]

    # gather-block order: chunk-major, then half, then tile within chunk
    blk_off = np.zeros((TPC, 2), dtype=np.int64)
    gather_blk0 = np.zeros((NCHUNKS, 2), dtype=np.int64)
    gather_nblk = np.zeros((NCHUNKS, 2), dtype=np.int64)
    pos = 0
    for c, (t0, t1) in enumerate(SPANS):
        for h in range(2):
            gather_blk0[c, h] = pos
            for t in range(t0, t1):
                blk_off[t, h] = pos
                pos += nb[t, h]
            gather_nblk[c, h] = pos - gather_blk0[c, h]
    totblk = pos

    # S-block order: chunk-major; self blocks of the chunk's tiles first,
    # then the chunk's gather blocks in gather order.
    s_off_self = np.zeros(TPC, dtype=np.int64)
    s_off_gblk = np.zeros(totblk, dtype=np.int64)
    s_chunk0 = np.zeros(NCHUNKS + 1, dtype=np.int64)
    spos = 0
    for c, (t0, t1) in enumerate(SPANS):
        s_chunk0[c] = spos
        for t in range(t0, t1):
            s_off_self[t] = spos
            spos += 1
        for h in range(2):
            for t in range(t0, t1):
                for j in range(int(nb[t, h])):
                    s_off_gblk[blk_off[t, h] + j] = spos
                    spos += 1
    s_chunk0[NCHUNKS] = spos
    tot_sblk = spos

    # per-unique-slot placement
    u_core = u_key // (TPC * 2)
    u_tile = (u_key // 2) % TPC
    u_half = u_key % 2
    u_blk = blk_off[u_tile, u_half] + urank // P
    u_slot = urank % P

    IDXALL = np.zeros((NCORES, totblk * P), dtype=np.int16)
    IDXALL[u_core, u_blk * P + u_slot] = u_src.astype(np.int16)

    # per-edge S entries (scatter-add: multi-hot rows for duplicate srcs)
    SBLK = np.zeros((NCORES, P, tot_sblk, P), dtype=np.float32)
    np.add.at(
        SBLK,
        (u_core[inv], u_slot[inv], s_off_gblk[u_blk[inv]], dstloc),
        coef,
    )

    # self blocks: diagonal coef for the tile's own nodes
    selfcoef = np.zeros(NPAD, dtype=np.float32)
    selfcoef[:N] = dinv[:N] * dinv[:N]
    sc = selfcoef.reshape(NCORES, TPC, P)
    ar = np.arange(P)
    for cc in range(NCORES):
        for t in range(TPC):
            SBLK[cc, ar, s_off_self[t], ar] = sc[cc, t]

    S_D = np.ascontiguousarray(
        SBLK.reshape(NCORES, P, tot_sblk * P)
    ).astype(ml_dtypes.bfloat16)

    # idx SBUF layout: pos k -> [k % 16, k // 16], replicated 8x over partitions
    IDX = np.tile(IDXALL.reshape(NCORES, -1, 16).transpose(0, 2, 1), (1, 8, 1))

    x_pad = np.zeros((NPAD, C), dtype=ml_dtypes.bfloat16)
    x_pad[:N] = x.astype(ml_dtypes.bfloat16)

    # per-core resident copy of the core's own dst rows, [128, TPC*128]
    xs = x_pad.reshape(NCORES, TPC, P, C).transpose(0, 2, 1, 3).reshape(
        NCORES, P, TPC * C
    )
    X_SELF = np.ascontiguousarray(xs)

    meta = dict(
        nb=nb,
        blk_off=blk_off,
        gather_blk0=gather_blk0,
        gather_nblk=gather_nblk,
        totblk=totblk,
        s_off_self=s_off_self,
        s_off_gblk=s_off_gblk,
        s_chunk0=s_chunk0,
        tot_sblk=tot_sblk,
    )
    return x_pad, IDX, S_D, X_SELF, WnT, b.reshape(1, C), meta


def _build(meta):
    """Build the SPMD Bass graph (shared across all 8 cores)."""
    nb = meta["nb"]
    blk_off = meta["blk_off"]
    gather_blk0 = meta["gather_blk0"]
    gather_nblk = meta["gather_nblk"]
    totblk = meta["totblk"]
    s_off_self = meta["s_off_self"]
    s_off_gblk = meta["s_off_gblk"]
    s_chunk0 = meta["s_chunk0"]
    tot_sblk = meta["tot_sblk"]

    nc = bacc.Bacc(
        "TRN2", target_bir_lowering=False, debug=False, num_swdge_queues=2
    )

    x_d = nc.dram_tensor("x", [NPAD, C], mybir.dt.bfloat16, kind="ExternalInput")
    idx_d = nc.dram_tensor("idx", [P, totblk * 8], mybir.dt.int16, kind="ExternalInput")
    s_d = nc.dram_tensor("s", [P, tot_sblk * P], mybir.dt.bfloat16, kind="ExternalInput")
    xself_d = nc.dram_tensor("xself", [P, TPC * C], mybir.dt.bfloat16, kind="ExternalInput")
    wnT_d = nc.dram_tensor("wnT", [C, C], mybir.dt.float32, kind="ExternalInput")
    b_d = nc.dram_tensor("b", [1, C], mybir.dt.float32, kind="ExternalInput")
    out_d = nc.dram_tensor("out", [NPC, C], mybir.dt.float32, kind="ExternalOutput")

    nbc_max = int(gather_nblk.sum(axis=1).max())
    nsc_max = int((s_chunk0[1:] - s_chunk0[:-1]).max())

    with tile.TileContext(nc) as tc, ExitStack() as ctx:
        meta_p = ctx.enter_context(tc.tile_pool(name="meta", bufs=1))
        idx_p = ctx.enter_context(tc.tile_pool(name="idx", bufs=3))
        gat_p = ctx.enter_context(tc.tile_pool(name="gat", bufs=4))
        s_p = ctx.enter_context(tc.tile_pool(name="s", bufs=3))
        agg_p = ctx.enter_context(tc.tile_pool(name="agg", bufs=4))
        out_p = ctx.enter_context(tc.tile_pool(name="out", bufs=4))
        small_p = ctx.enter_context(tc.tile_pool(name="small", bufs=8))
        ps_agg = ctx.enter_context(tc.tile_pool(name="ps_agg", bufs=4, space="PSUM"))
        ps_out = ctx.enter_context(tc.tile_pool(name="ps_out", bufs=3, space="PSUM"))

        # warm-up: tiny gather to overlap the GPSIMD ucode IRAM load with the
        # initial meta DMAs (first SWDGE call pays ~6us otherwise)
        warm_idx = meta_p.tile([P, 8], mybir.dt.int16)
        nc.vector.memset(warm_idx[:], 0)
        warm_out = meta_p.tile([P, 1, P], mybir.dt.bfloat16)
        nc.gpsimd.dma_gather(
            out_ap=warm_out[:],
            in_ap=x_d[0:HALF, :],
            idxs_ap=warm_idx[:],
            num_idxs=P,
            num_idxs_reg=P,
            elem_size=C,
            single_packet=False,
        )

        x_lo = x_d[0:HALF, :]
        x_hi = x_d[HALF:NPAD, :]

        nbi_max = int((gather_nblk[:, 0] + gather_nblk[:, 1]).max())

        def issue_gather(ci):
            cblk0 = int(gather_blk0[ci, 0])
            nblk_c = int(gather_nblk[ci, 0] + gather_nblk[ci, 1])
            csblk0 = int(s_chunk0[ci])
            ns_c = int(s_chunk0[ci + 1]) - csblk0

            idx_sb = idx_p.tile([P, nbi_max * 8], mybir.dt.int16, tag="idx")
            nc.sync.dma_start(
                idx_sb[:, : nblk_c * 8], idx_d[:, cblk0 * 8 : (cblk0 + nblk_c) * 8]
            )

            s_sb = s_p.tile([P, nsc_max * P], mybir.dt.bfloat16, tag="s")
            nc.sync.dma_start(
                s_sb[:, : ns_c * P], s_d[:, csblk0 * P : (csblk0 + ns_c) * P]
            )

            gat_sb = gat_p.tile([P, nbc_max, P], mybir.dt.bfloat16, tag="gat")
            for h, src_ap in ((0, x_lo), (1, x_hi)):
                nblk_g = int(gather_nblk[ci, h])
                if nblk_g == 0:
                    continue
                nidx = nblk_g * P
                g0 = int(gather_blk0[ci, h]) - cblk0
                nc.gpsimd.dma_gather(
                    out_ap=gat_sb[:, g0 : g0 + nblk_g, :],
                    in_ap=src_ap,
                    idxs_ap=idx_sb[:, g0 * 8 : g0 * 8 + nidx // 16],
                    num_idxs=nidx,
                    num_idxs_reg=nidx,
                    elem_size=C,
                    single_packet=False,
                    queue_num=(ci + h) % 2,
                )
            return s_sb, gat_sb

        # chunk 0's gather goes first in program order so its DMA-sem lane
        # isn't queued behind the large meta loads
        pending = issue_gather(0)

        xself_sb = meta_p.tile([P, TPC * C], mybir.dt.bfloat16)
        nc.sync.dma_start(xself_sb[:], xself_d[:])
        wnT_sb = meta_p.tile([C, C], mybir.dt.float32)
        nc.sync.dma_start(wnT_sb[:], wnT_d[:])
        b_sb = meta_p.tile([1, C], mybir.dt.float32)
        nc.sync.dma_start(b_sb[:], b_d[:])
        ones_sb = meta_p.tile([1, C], mybir.dt.float32)
        nc.vector.memset(ones_sb[:], 1.0)
        eps_sb = meta_p.tile([P, 1], mybir.dt.float32)
        nc.vector.memset(eps_sb[:], EPS_IN)

        for ci, (t0, t1) in enumerate(SPANS):
            s_sb, gat_sb = pending
            if ci + 1 < NCHUNKS:
                pending = issue_gather(ci + 1)
            cblk0 = int(gather_blk0[ci, 0])
            csblk0 = int(s_chunk0[ci])

            for t in range(t0, t1):
                # (lhsT source, S block index within chunk) accumulation list
                mms = [("self", int(s_off_self[t]) - csblk0)]
                for h in range(2):
                    for j in range(int(nb[t, h])):
                        g = int(blk_off[t, h]) + j
                        mms.append((g - cblk0, int(s_off_gblk[g]) - csblk0))

                pt = ps_agg.tile([P, P], mybir.dt.float32)
                for j, (lsrc, sc_col) in enumerate(mms):
                    if lsrc == "self":
                        lhsT = xself_sb[:, t * C : (t + 1) * C]
                    else:
                        lhsT = gat_sb[:, lsrc, :]
                    nc.tensor.matmul(
                        pt[:],
                        lhsT=lhsT,
                        rhs=s_sb[:, sc_col * P : (sc_col + 1) * P],
                        start=(j == 0),
                        stop=(j == len(mms) - 1),
                    )

                agg_sb = agg_p.tile([P, P], mybir.dt.float32)
                nc.scalar.copy(agg_sb[:], pt[:])

                po = ps_out.tile([P, P], mybir.dt.float32)
                nc.tensor.matmul(po[:], lhsT=agg_sb[:], rhs=wnT_sb[:], start=True, stop=False)
                nc.tensor.matmul(po[:], lhsT=ones_sb[:], rhs=b_sb[:], start=False, stop=True)

                # InstanceNorm + LeakyReLU: stats on DVE (small), the full-tile
                # normalize+activate fused into one ACT Lrelu op.
                stats = small_p.tile([P, 6], mybir.dt.float32)
                nc.vector.bn_stats(out=stats[:], in_=po[:])
                mv = small_p.tile([P, 2], mybir.dt.float32)
                nc.vector.bn_aggr(out=mv[:], in_=stats[:])
                std = small_p.tile([P, 1], mybir.dt.float32)
                nc.scalar.activation(
                    out=std[:], in_=mv[:, 1:2],
                    func=mybir.ActivationFunctionType.Sqrt,
                    bias=eps_sb[:], scale=1.0,
                )
                rstd = small_p.tile([P, 1], mybir.dt.float32)
                nc.vector.reciprocal(out=rstd[:], in_=std[:])
                nbias = small_p.tile([P, 1], mybir.dt.float32)
                nc.vector.tensor_scalar(
                    out=nbias[:], in0=mv[:, 0:1], scalar1=rstd[:], scalar2=-1.0,
                    op0=mybir.AluOpType.mult, op1=mybir.AluOpType.mult,
                )
                y_sb = out_p.tile([P, P], mybir.dt.float32, tag="y")
                nc.scalar.activation(
                    out=y_sb[:], in_=po[:],
                    func=mybir.ActivationFunctionType.Identity,
                    bias=nbias[:], scale=rstd[:],
                )
                final = out_p.tile([P, P], mybir.dt.float32, tag="final")
                nc.vector.scalar_tensor_tensor(
                    out=final[:], in0=y_sb[:], scalar=0.2, in1=y_sb[:],
                    op0=mybir.AluOpType.mult, op1=mybir.AluOpType.max,
                )
                nc.sync.dma_start(out_d[t * P : (t + 1) * P, :], final[:])

    nc.compile()
    return nc


def _make_in_maps(x_pad, IDX, S_D, X_SELF, WnT, bvec):
    return [
        {
            "x": x_pad,
            "idx": np.ascontiguousarray(IDX[i]),
            "s": np.ascontiguousarray(S_D[i]),
            "xself": np.ascontiguousarray(X_SELF[i]),
            "wnT": WnT,
            "b": bvec,
        }
        for i in range(NCORES)
    ]


def kernel(x, edge_index, W, b, u):
    x_pad, IDX, S_D, X_SELF, WnT, bvec, meta = _preprocess(x, edge_index, W, b, u)
    nc = _build(meta)
    in_maps = _make_in_maps(x_pad, IDX, S_D, X_SELF, WnT, bvec)

    # The axon terminal can be transiently unavailable right after a prior
    # process's teardown; retry with backoff.
    import time

    last_err = None
    for attempt in range(6):
        try:
            res = run_bass_kernel_spmd(nc, in_maps, list(range(NCORES)))
            break
        except Exception as e:  # noqa: BLE001
            last_err = e
            time.sleep(45)
    else:
        raise last_err
    shards = [np.asarray(res.results[i]["out"]) for i in range(NCORES)]
    out = np.concatenate(shards, axis=0)[:N]
    return out.astype(np.float32)


# revision 16
# speedup vs baseline: 1.6607x; 1.0305x over previous
"""Trainium2 Bass kernel for GCNBlock (spectral-norm linear + GCN aggregation +
InstanceNorm + LeakyReLU) distributed across 8 NeuronCores.

Strategy (per the dst-sharding hint):
  - out = A @ (x @ WnT) = (A @ x) @ WnT: aggregate raw x rows first, then
    apply the 128x128 weight per dst tile.
  - dst nodes sharded across 8 cores (6272 nodes = 49 tiles of 128 per core).
    Real edges partitioned by dst, grouped per (tile, src-half), DEDUPED per
    group (duplicate srcs share a gather slot; S rows become multi-hot),
    padded to 128-slot blocks (src halves because dma_gather indices are
    int16).  Self loops are folded in via a resident x tile + diagonal S.
  - The scatter matrices S[slot, dst] = coef * onehot are precomputed on the
    host and streamed in bf16 via HWDGE.  The serial bottleneck is GPSIMD's
    SWDGE descriptor emission for the gathers (~8 ns/row, measured); all
    other engines are kept quiet to avoid inflating it -- in particular the
    out-stage runs on the Scalar engine (one fused Lrelu activation) because
    DVE shares its SBUF port with GPSIMD.
  - Per block: bulk-gather 128 src rows of x (bf16) via dma_gather, then
    aggT[cin, dst] += Xsrc.T @ S on the PE in PSUM.
  - Per dst tile: out = aggT.T @ WnT + b (PE), InstanceNorm via
    bn_stats/bn_aggr, then one ACT Lrelu(scale*x+bias, alpha=0.2), DMA out.
"""

import numpy as np
import ml_dtypes
from contextlib import ExitStack

import concourse.tile as tile
from concourse import bacc, mybir
from concourse.bass_utils import run_bass_kernel_spmd

# Problem constants (hardcoded per spec)
N, E, C = 50000, 800000, 128
P = 128
NCORES = 8
TPC = 49                # dst tiles per core
NPC = TPC * P           # 6272 dst nodes per core
NPAD = NCORES * NPC     # 50176 padded node count
HALF = 32768            # int16 index split point
# chunk spans of dst tiles; last chunk is a single tile to shrink the tail
SPANS = [(i * 4, i * 4 + 4) for i in range(12)] + [(48, 49)]
NCHUNKS = len(SPANS)
EPS_IN = 1e-5


def _preprocess(x, edge_index, W, b, u):
    """Host-side prep: spectral norm, edge partitioning, S-matrix layout."""
    x = np.asarray(x, dtype=np.float32)
    ei = np.asarray(edge_index)
    W = np.asarray(W, dtype=np.float32)
    b = np.asarray(b, dtype=np.float32)
    u = np.asarray(u, dtype=np.float32)

    # --- spectral norm (one power iteration), matches reference ---
    eps = np.float32(1e-12)
    v = (W.T @ u).astype(np.float32)
    v = v / (np.float32(np.linalg.norm(v)) + eps)
    Wv = (W @ v).astype(np.float32)
    u2 = Wv / (np.float32(np.linalg.norm(Wv)) + eps)
    sigma = np.float32(u2 @ Wv)
    WnT = np.ascontiguousarray((W / sigma).T, dtype=np.float32)  # [cin, cout]

    src = ei[0].astype(np.int64)
    dst = ei[1].astype(np.int64)

    # --- degrees / coefficients (self loops in degree only) ---
    deg = (np.bincount(dst, minlength=N) + 1).astype(np.float32)
    dinv = (1.0 / np.sqrt(deg)).astype(np.float32)
    coef = dinv[src] * dinv[dst]

    core = dst // NPC
    tile_g = (dst % NPC) // P
    dstloc = dst % P
    half = (src >= HALF).astype(np.int64)

    # --- dedup: gather slot key is (core, tile, half, src) ---
    key = ((core * TPC + tile_g) * 2 + half).astype(np.int64)
    NG = NCORES * TPC * 2
    slotkey = key * 65536 + (src - half * HALF)
    uniq, inv = np.unique(slotkey, return_inverse=True)
    u_key = (uniq // 65536).astype(np.int64)
    u_src = (uniq % 65536).astype(np.int64)
    ucounts = np.bincount(u_key, minlength=NG)
    ustarts = np.zeros(NG + 1, dtype=np.int64)
    np.cumsum(ucounts, out=ustarts[1:])
    urank = np.arange(len(uniq), dtype=np.int64) - ustarts[u_key]

    cnt3 = ucounts.reshape(NCORES, TPC, 2)
    nb = np.ceil(cnt3.max(axis=0) / P).astype(np.int64)  # [TPC, 2]

    # gather-block order: chunk-major, then half, then tile within chunk
    blk_off = np.zeros((TPC, 2), dtype=np.int64)
    gather_blk0 = np.zeros((NCHUNKS, 2), dtype=np.int64)
    gather_nblk = np.zeros((NCHUNKS, 2), dtype=np.int64)
    pos = 0
    for c, (t0, t1) in enumerate(SPANS):
        for h in range(2):
            gather_blk0[c, h] = pos
            for t in range(t0, t1):
                blk_off[t, h] = pos
                pos += nb[t, h]
            gather_nblk[c, h] = pos - gather_blk0[c, h]
    totblk = pos

    # S-block order: chunk-major; self blocks of the chunk's tiles first,
    # then the chunk's gather blocks in gather order.
    s_off_self = np.zeros(TPC, dtype=np.int64)
    s_off_gblk = np.zeros(totblk, dtype=np.int64)
    s_chunk0 = np.zeros(NCHUNKS + 1, dtype=np.int64)
    spos = 0
    for c, (t0, t1) in enumerate(SPANS):
        s_chunk0[c] = spos
        for t in range(t0, t1):
            s_off_self[t] = spos
            spos += 1
        for h in range(2):
            for t in range(t0, t1):
                for j in range(int(nb[t, h])):
                    s_off_gblk[blk_off[t, h] + j] = spos
                    spos += 1
    s_chunk0[NCHUNKS] = spos
    tot_sblk = spos

    # per-unique-slot placement
    u_core = u_key // (TPC * 2)
    u_tile = (u_key // 2) % TPC
    u_half = u_key % 2
    u_blk = blk_off[u_tile, u_half] + urank // P
    u_slot = urank % P

    IDXALL = np.zeros((NCORES, totblk * P), dtype=np.int16)
    IDXALL[u_core, u_blk * P + u_slot] = u_src.astype(np.int16)

    # per-edge S entries (scatter-add: multi-hot rows for duplicate srcs)
    SBLK = np.zeros((NCORES, P, tot_sblk, P), dtype=np.float32)
    np.add.at(
        SBLK,
        (u_core[inv], u_slot[inv], s_off_gblk[u_blk[inv]], dstloc),
        coef,
    )

    # self blocks: diagonal coef for the tile's own nodes
    selfcoef = np.zeros(NPAD, dtype=np.float32)
    selfcoef[:N] = dinv[:N] * dinv[:N]
    sc = selfcoef.reshape(NCORES, TPC, P)
    ar = np.arange(P)
    for cc in range(NCORES):
        for t in range(TPC):
            SBLK[cc, ar, s_off_self[t], ar] = sc[cc, t]

    S_D = np.ascontiguousarray(
        SBLK.reshape(NCORES, P, tot_sblk * P)
    ).astype(ml_dtypes.bfloat16)

    # idx SBUF layout: pos k -> [k % 16, k // 16], replicated 8x over partitions
    IDX = np.tile(IDXALL.reshape(NCORES, -1, 16).transpose(0, 2, 1), (1, 8, 1))

    x_pad = np.zeros((NPAD, C), dtype=ml_dtypes.bfloat16)
    x_pad[:N] = x.astype(ml_dtypes.bfloat16)

    # per-core resident copy of the core's own dst rows, [128, TPC*128]
    xs = x_pad.reshape(NCORES, TPC, P, C).transpose(0, 2, 1, 3).reshape(
        NCORES, P, TPC * C
    )
    X_SELF = np.ascontiguousarray(xs)

    meta = dict(
        nb=nb,
        blk_off=blk_off,
        gather_blk0=gather_blk0,
        gather_nblk=gather_nblk,
        totblk=totblk,
        s_off_self=s_off_self,
        s_off_gblk=s_off_gblk,
        s_chunk0=s_chunk0,
        tot_sblk=tot_sblk,
    )
    return x_pad, IDX, S_D, X_SELF, WnT, b.reshape(1, C), meta


def _build(meta):
    """Build the SPMD Bass graph (shared across all 8 cores)."""
    nb = meta["nb"]
    blk_off = meta["blk_off"]
    gather_blk0 = meta["gather_blk0"]
    gather_nblk = meta["gather_nblk"]
    totblk = meta["totblk"]
    s_off_self = meta["s_off_self"]
    s_off_gblk = meta["s_off_gblk"]
    s_chunk0 = meta["s_chunk0"]
    tot_sblk = meta["tot_sblk"]

    nc = bacc.Bacc(
        "TRN2", target_bir_lowering=False, debug=False, num_swdge_queues=4
    )

    x_d = nc.dram_tensor("x", [NPAD, C], mybir.dt.bfloat16, kind="ExternalInput")
    idx_d = nc.dram_tensor("idx", [P, totblk * 8], mybir.dt.int16, kind="ExternalInput")
    s_d = nc.dram_tensor("s", [P, tot_sblk * P], mybir.dt.bfloat16, kind="ExternalInput")
    xself_d = nc.dram_tensor("xself", [P, TPC * C], mybir.dt.bfloat16, kind="ExternalInput")
    wnT_d = nc.dram_tensor("wnT", [C, C], mybir.dt.float32, kind="ExternalInput")
    b_d = nc.dram_tensor("b", [1, C], mybir.dt.float32, kind="ExternalInput")
    out_d = nc.dram_tensor("out", [NPC, C], mybir.dt.float32, kind="ExternalOutput")

    nbc_max = int(gather_nblk.sum(axis=1).max())
    nsc_max = int((s_chunk0[1:] - s_chunk0[:-1]).max())

    with tile.TileContext(nc) as tc, ExitStack() as ctx:
        meta_p = ctx.enter_context(tc.tile_pool(name="meta", bufs=1))
        idx_p = ctx.enter_context(tc.tile_pool(name="idx", bufs=3))
        gat_p = ctx.enter_context(tc.tile_pool(name="gat", bufs=4))
        s_p = ctx.enter_context(tc.tile_pool(name="s", bufs=3))
        agg_p = ctx.enter_context(tc.tile_pool(name="agg", bufs=4))
        out_p = ctx.enter_context(tc.tile_pool(name="out", bufs=4))
        small_p = ctx.enter_context(tc.tile_pool(name="small", bufs=8))
        ps_agg = ctx.enter_context(tc.tile_pool(name="ps_agg", bufs=4, space="PSUM"))
        ps_out = ctx.enter_context(tc.tile_pool(name="ps_out", bufs=3, space="PSUM"))

        # warm-up: tiny gather to overlap the GPSIMD ucode IRAM load with the
        # initial meta DMAs (first SWDGE call pays ~6us otherwise)
        warm_idx = meta_p.tile([P, 8], mybir.dt.int16)
        nc.vector.memset(warm_idx[:], 0)
        warm_out = meta_p.tile([P, 1, P], mybir.dt.bfloat16)
        nc.gpsimd.dma_gather(
            out_ap=warm_out[:],
            in_ap=x_d[0:HALF, :],
            idxs_ap=warm_idx[:],
            num_idxs=P,
            num_idxs_reg=P,
            elem_size=C,
            single_packet=False,
        )

        x_lo = x_d[0:HALF, :]
        x_hi = x_d[HALF:NPAD, :]

        nbi_max = int((gather_nblk[:, 0] + gather_nblk[:, 1]).max())

        def issue_gather(ci):
            cblk0 = int(gather_blk0[ci, 0])
            nblk_c = int(gather_nblk[ci, 0] + gather_nblk[ci, 1])
            csblk0 = int(s_chunk0[ci])
            ns_c = int(s_chunk0[ci + 1]) - csblk0

            idx_sb = idx_p.tile([P, nbi_max * 8], mybir.dt.int16, tag="idx")
            nc.sync.dma_start(
                idx_sb[:, : nblk_c * 8], idx_d[:, cblk0 * 8 : (cblk0 + nblk_c) * 8]
            )

            s_sb = s_p.tile([P, nsc_max * P], mybir.dt.bfloat16, tag="s")
            nc.sync.dma_start(
                s_sb[:, : ns_c * P], s_d[:, csblk0 * P : (csblk0 + ns_c) * P]
            )

            gat_sb = gat_p.tile([P, nbc_max, P], mybir.dt.bfloat16, tag="gat")
            for h, src_ap in ((0, x_lo), (1, x_hi)):
                nblk_g = int(gather_nblk[ci, h])
                if nblk_g == 0:
                    continue
                nidx = nblk_g * P
                g0 = int(gather_blk0[ci, h]) - cblk0
                nc.gpsimd.dma_gather(
                    out_ap=gat_sb[:, g0 : g0 + nblk_g, :],
                    in_ap=src_ap,
                    idxs_ap=idx_sb[:, g0 * 8 : g0 * 8 + nidx // 16],
                    num_idxs=nidx,
                    num_idxs_reg=nidx,
                    elem_size=C,
                    single_packet=False,
                    queue_num=(ci + h) % 4,
                )
            return s_sb, gat_sb

        # chunk 0's gather goes first in program order so its DMA-sem lane
        # isn't queued behind the large meta loads
        pending = issue_gather(0)

        xself_sb = meta_p.tile([P, TPC * C], mybir.dt.bfloat16)
        nc.sync.dma_start(xself_sb[:], xself_d[:])
        wnT_sb = meta_p.tile([C, C], mybir.dt.float32)
        nc.sync.dma_start(wnT_sb[:], wnT_d[:])
        b_sb = meta_p.tile([1, C], mybir.dt.float32)
        nc.sync.dma_start(b_sb[:], b_d[:])
        ones_sb = meta_p.tile([1, C], mybir.dt.float32)
        nc.vector.memset(ones_sb[:], 1.0)
        eps_sb = meta_p.tile([P, 1], mybir.dt.float32)
        nc.vector.memset(eps_sb[:], EPS_IN)

        for ci, (t0, t1) in enumerate(SPANS):
            s_sb, gat_sb = pending
            if ci + 1 < NCHUNKS:
                pending = issue_gather(ci + 1)
            cblk0 = int(gather_blk0[ci, 0])
            csblk0 = int(s_chunk0[ci])

            for t in range(t0, t1):
                # (lhsT source, S block index within chunk) accumulation list
                mms = [("self", int(s_off_self[t]) - csblk0)]
                for h in range(2):
                    for j in range(int(nb[t, h])):
                        g = int(blk_off[t, h]) + j
                        mms.append((g - cblk0, int(s_off_gblk[g]) - csblk0))

                pt = ps_agg.tile([P, P], mybir.dt.float32)
                for j, (lsrc, sc_col) in enumerate(mms):
                    if lsrc == "self":
                        lhsT = xself_sb[:, t * C : (t + 1) * C]
                    else:
                        lhsT = gat_sb[:, lsrc, :]
                    nc.tensor.matmul(
                        pt[:],
                        lhsT=lhsT,
                        rhs=s_sb[:, sc_col * P : (sc_col + 1) * P],
                        start=(j == 0),
                        stop=(j == len(mms) - 1),
                    )

                agg_sb = agg_p.tile([P, P], mybir.dt.float32)
                nc.scalar.copy(agg_sb[:], pt[:])

                po = ps_out.tile([P, P], mybir.dt.float32)
                nc.tensor.matmul(po[:], lhsT=agg_sb[:], rhs=wnT_sb[:], start=True, stop=False)
                nc.tensor.matmul(po[:], lhsT=ones_sb[:], rhs=b_sb[:], start=False, stop=True)

                # InstanceNorm + LeakyReLU: stats on DVE (small), the full-tile
                # normalize+activate fused into one ACT Lrelu op.
                stats = small_p.tile([P, 6], mybir.dt.float32)
                nc.vector.bn_stats(out=stats[:], in_=po[:])
                mv = small_p.tile([P, 2], mybir.dt.float32)
                nc.vector.bn_aggr(out=mv[:], in_=stats[:])
                std = small_p.tile([P, 1], mybir.dt.float32)
                nc.scalar.activation(
                    out=std[:], in_=mv[:, 1:2],
                    func=mybir.ActivationFunctionType.Sqrt,
                    bias=eps_sb[:], scale=1.0,
                )
                rstd = small_p.tile([P, 1], mybir.dt.float32)
                nc.vector.reciprocal(out=rstd[:], in_=std[:])
                nbias = small_p.tile([P, 1], mybir.dt.float32)
                nc.vector.tensor_scalar(
                    out=nbias[:], in0=mv[:, 0:1], scalar1=rstd[:], scalar2=-1.0,
                    op0=mybir.AluOpType.mult, op1=mybir.AluOpType.mult,
                )
                y_sb = out_p.tile([P, P], mybir.dt.float32, tag="y")
                nc.scalar.activation(
                    out=y_sb[:], in_=po[:],
                    func=mybir.ActivationFunctionType.Identity,
                    bias=nbias[:], scale=rstd[:],
                )
                final = out_p.tile([P, P], mybir.dt.float32, tag="final")
                nc.vector.scalar_tensor_tensor(
                    out=final[:], in0=y_sb[:], scalar=0.2, in1=y_sb[:],
                    op0=mybir.AluOpType.mult, op1=mybir.AluOpType.max,
                )
                nc.sync.dma_start(out_d[t * P : (t + 1) * P, :], final[:])

    nc.compile()
    return nc


def _make_in_maps(x_pad, IDX, S_D, X_SELF, WnT, bvec):
    return [
        {
            "x": x_pad,
            "idx": np.ascontiguousarray(IDX[i]),
            "s": np.ascontiguousarray(S_D[i]),
            "xself": np.ascontiguousarray(X_SELF[i]),
            "wnT": WnT,
            "b": bvec,
        }
        for i in range(NCORES)
    ]


def kernel(x, edge_index, W, b, u):
    x_pad, IDX, S_D, X_SELF, WnT, bvec, meta = _preprocess(x, edge_index, W, b, u)
    nc = _build(meta)
    in_maps = _make_in_maps(x_pad, IDX, S_D, X_SELF, WnT, bvec)

    # The axon terminal can be transiently unavailable right after a prior
    # process's teardown; retry with backoff.
    import time

    last_err = None
    for attempt in range(6):
        try:
            res = run_bass_kernel_spmd(nc, in_maps, list(range(NCORES)))
            break
        except Exception as e:  # noqa: BLE001
            last_err = e
            time.sleep(45)
    else:
        raise last_err
    shards = [np.asarray(res.results[i]["out"]) for i in range(NCORES)]
    out = np.concatenate(shards, axis=0)[:N]
    return out.astype(np.float32)


# revision 18
# speedup vs baseline: 1.8037x; 1.0861x over previous
"""Trainium2 Bass kernel for GCNBlock (spectral-norm linear + GCN aggregation +
InstanceNorm + LeakyReLU) distributed across 8 NeuronCores.

Strategy (per the dst-sharding hint):
  - out = A @ (x @ WnT) = (A @ x) @ WnT: aggregate raw x rows first, then
    apply the 128x128 weight per dst tile.
  - dst nodes sharded across 8 cores (6272 nodes = 49 tiles of 128 per core).
    Real edges partitioned by dst, grouped per (tile, src-half), DEDUPED per
    group (duplicate srcs share a gather slot; S rows become multi-hot),
    padded to 128-slot blocks (src halves because dma_gather indices are
    int16).  Self loops are folded in via a resident x tile + diagonal S.
  - The scatter matrices S[slot, dst] = coef * onehot are precomputed on the
    host and streamed in bf16 via HWDGE.  The serial bottleneck is GPSIMD's
    SWDGE descriptor emission for the gathers (~8 ns/row, measured); all
    other engines are kept quiet to avoid inflating it -- in particular the
    out-stage runs on the Scalar engine (one fused Lrelu activation) because
    DVE shares its SBUF port with GPSIMD.
  - Per block: bulk-gather 128 src rows of x (bf16) via dma_gather, then
    aggT[cin, dst] += Xsrc.T @ S on the PE in PSUM.
  - Per dst tile: out = aggT.T @ WnT + b (PE), InstanceNorm via
    bn_stats/bn_aggr, then one ACT Lrelu(scale*x+bias, alpha=0.2), DMA out.
"""

import numpy as np
import ml_dtypes
from contextlib import ExitStack

import concourse.tile as tile
from concourse import bacc, mybir
from concourse.bass_utils import run_bass_kernel_spmd

# Problem constants (hardcoded per spec)
N, E, C = 50000, 800000, 128
P = 128
NCORES = 8
TPC = 49                # dst tiles per core
NPC = TPC * P           # 6272 dst nodes per core
NPAD = NCORES * NPC     # 50176 padded node count
HALF = 32768            # int16 index split point
# chunk spans of dst tiles; small chunks keep the compute backlog after the
# final gather short (queue-detached emission hides per-call fixed costs)
SPANS = [(i * 2, i * 2 + 2) for i in range(24)] + [(48, 49)]
NCHUNKS = len(SPANS)
EPS_IN = 1e-5


def _preprocess(x, edge_index, W, b, u):
    """Host-side prep: spectral norm, edge partitioning, S-matrix layout."""
    x = np.asarray(x, dtype=np.float32)
    ei = np.asarray(edge_index)
    W = np.asarray(W, dtype=np.float32)
    b = np.asarray(b, dtype=np.float32)
    u = np.asarray(u, dtype=np.float32)

    # --- spectral norm (one power iteration), matches reference ---
    eps = np.float32(1e-12)
    v = (W.T @ u).astype(np.float32)
    v = v / (np.float32(np.linalg.norm(v)) + eps)
    Wv = (W @ v).astype(np.float32)
    u2 = Wv / (np.float32(np.linalg.norm(Wv)) + eps)
    sigma = np.float32(u2 @ Wv)
    WnT = np.ascontiguousarray((W / sigma).T, dtype=np.float32)  # [cin, cout]

    src = ei[0].astype(np.int64)
    dst = ei[1].astype(np.int64)

    # --- degrees / coefficients (self loops in degree only) ---
    deg = (np.bincount(dst, minlength=N) + 1).astype(np.float32)
    dinv = (1.0 / np.sqrt(deg)).astype(np.float32)
    coef = dinv[src] * dinv[dst]

    core = dst // NPC
    tile_g = (dst % NPC) // P
    dstloc = dst % P
    half = (src >= HALF).astype(np.int64)

    # --- dedup: gather slot key is (core, tile, half, src) ---
    key = ((core * TPC + tile_g) * 2 + half).astype(np.int64)
    NG = NCORES * TPC * 2
    slotkey = key * 65536 + (src - half * HALF)
    uniq, inv = np.unique(slotkey, return_inverse=True)
    u_key = (uniq // 65536).astype(np.int64)
    u_src = (uniq % 65536).astype(np.int64)
    ucounts = np.bincount(u_key, minlength=NG)
    ustarts = np.zeros(NG + 1, dtype=np.int64)
    np.cumsum(ucounts, out=ustarts[1:])
    urank = np.arange(len(uniq), dtype=np.int64) - ustarts[u_key]

    cnt3 = ucounts.reshape(NCORES, TPC, 2)
    nb = np.ceil(cnt3.max(axis=0) / P).astype(np.int64)  # [TPC, 2]

    # gather-block order: chunk-major, then half, then tile within chunk
    blk_off = np.zeros((TPC, 2), dtype=np.int64)
    gather_blk0 = np.zeros((NCHUNKS, 2), dtype=np.int64)
    gather_nblk = np.zeros((NCHUNKS, 2), dtype=np.int64)
    pos = 0
    for c, (t0, t1) in enumerate(SPANS):
        for h in range(2):
            gather_blk0[c, h] = pos
            for t in range(t0, t1):
                blk_off[t, h] = pos
                pos += nb[t, h]
            gather_nblk[c, h] = pos - gather_blk0[c, h]
    totblk = pos

    # S-block order: chunk-major; self blocks of the chunk's tiles first,
    # then the chunk's gather blocks in gather order.
    s_off_self = np.zeros(TPC, dtype=np.int64)
    s_off_gblk = np.zeros(totblk, dtype=np.int64)
    s_chunk0 = np.zeros(NCHUNKS + 1, dtype=np.int64)
    spos = 0
    for c, (t0, t1) in enumerate(SPANS):
        s_chunk0[c] = spos
        for t in range(t0, t1):
            s_off_self[t] = spos
            spos += 1
        for h in range(2):
            for t in range(t0, t1):
                for j in range(int(nb[t, h])):
                    s_off_gblk[blk_off[t, h] + j] = spos
                    spos += 1
    s_chunk0[NCHUNKS] = spos
    tot_sblk = spos

    # per-unique-slot placement
    u_core = u_key // (TPC * 2)
    u_tile = (u_key // 2) % TPC
    u_half = u_key % 2
    u_blk = blk_off[u_tile, u_half] + urank // P
    u_slot = urank % P

    IDXALL = np.zeros((NCORES, totblk * P), dtype=np.int16)
    IDXALL[u_core, u_blk * P + u_slot] = u_src.astype(np.int16)

    # per-edge S entries (scatter-add: multi-hot rows for duplicate srcs)
    SBLK = np.zeros((NCORES, P, tot_sblk, P), dtype=np.float32)
    np.add.at(
        SBLK,
        (u_core[inv], u_slot[inv], s_off_gblk[u_blk[inv]], dstloc),
        coef,
    )

    # self blocks: diagonal coef for the tile's own nodes
    selfcoef = np.zeros(NPAD, dtype=np.float32)
    selfcoef[:N] = dinv[:N] * dinv[:N]
    sc = selfcoef.reshape(NCORES, TPC, P)
    ar = np.arange(P)
    for cc in range(NCORES):
        for t in range(TPC):
            SBLK[cc, ar, s_off_self[t], ar] = sc[cc, t]

    S_D = np.ascontiguousarray(
        SBLK.reshape(NCORES, P, tot_sblk * P)
    ).astype(ml_dtypes.bfloat16)

    # idx SBUF layout: pos k -> [k % 16, k // 16], replicated 8x over partitions
    IDX = np.tile(IDXALL.reshape(NCORES, -1, 16).transpose(0, 2, 1), (1, 8, 1))

    x_pad = np.zeros((NPAD, C), dtype=ml_dtypes.bfloat16)
    x_pad[:N] = x.astype(ml_dtypes.bfloat16)

    # per-core resident copy of the core's own dst rows, [128, TPC*128]
    xs = x_pad.reshape(NCORES, TPC, P, C).transpose(0, 2, 1, 3).reshape(
        NCORES, P, TPC * C
    )
    X_SELF = np.ascontiguousarray(xs)

    meta = dict(
        nb=nb,
        blk_off=blk_off,
        gather_blk0=gather_blk0,
        gather_nblk=gather_nblk,
        totblk=totblk,
        s_off_self=s_off_self,
        s_off_gblk=s_off_gblk,
        s_chunk0=s_chunk0,
        tot_sblk=tot_sblk,
    )
    return x_pad, IDX, S_D, X_SELF, WnT, b.reshape(1, C), meta


def _build(meta):
    """Build the SPMD Bass graph (shared across all 8 cores)."""
    nb = meta["nb"]
    blk_off = meta["blk_off"]
    gather_blk0 = meta["gather_blk0"]
    gather_nblk = meta["gather_nblk"]
    totblk = meta["totblk"]
    s_off_self = meta["s_off_self"]
    s_off_gblk = meta["s_off_gblk"]
    s_chunk0 = meta["s_chunk0"]
    tot_sblk = meta["tot_sblk"]

    nc = bacc.Bacc(
        "TRN2", target_bir_lowering=False, debug=False, num_swdge_queues=4
    )

    x_d = nc.dram_tensor("x", [NPAD, C], mybir.dt.bfloat16, kind="ExternalInput")
    idx_d = nc.dram_tensor("idx", [P, totblk * 8], mybir.dt.int16, kind="ExternalInput")
    s_d = nc.dram_tensor("s", [P, tot_sblk * P], mybir.dt.bfloat16, kind="ExternalInput")
    xself_d = nc.dram_tensor("xself", [P, TPC * C], mybir.dt.bfloat16, kind="ExternalInput")
    wnT_d = nc.dram_tensor("wnT", [C, C], mybir.dt.float32, kind="ExternalInput")
    b_d = nc.dram_tensor("b", [1, C], mybir.dt.float32, kind="ExternalInput")
    out_d = nc.dram_tensor("out", [NPC, C], mybir.dt.float32, kind="ExternalOutput")

    nbc_max = int(gather_nblk.sum(axis=1).max())
    nsc_max = int((s_chunk0[1:] - s_chunk0[:-1]).max())

    with tile.TileContext(nc) as tc, ExitStack() as ctx:
        meta_p = ctx.enter_context(tc.tile_pool(name="meta", bufs=1))
        idx_p = ctx.enter_context(tc.tile_pool(name="idx", bufs=6))
        gat_p = ctx.enter_context(tc.tile_pool(name="gat", bufs=6))
        s_p = ctx.enter_context(tc.tile_pool(name="s", bufs=5))
        agg_p = ctx.enter_context(tc.tile_pool(name="agg", bufs=4))
        out_p = ctx.enter_context(tc.tile_pool(name="out", bufs=4))
        small_p = ctx.enter_context(tc.tile_pool(name="small", bufs=8))
        ps_agg = ctx.enter_context(tc.tile_pool(name="ps_agg", bufs=4, space="PSUM"))
        ps_out = ctx.enter_context(tc.tile_pool(name="ps_out", bufs=3, space="PSUM"))

        # warm-up: tiny gather to overlap the GPSIMD ucode IRAM load with the
        # initial meta DMAs (first SWDGE call pays ~6us otherwise)
        warm_idx = meta_p.tile([P, 8], mybir.dt.int16)
        nc.vector.memset(warm_idx[:], 0)
        warm_out = meta_p.tile([P, 1, P], mybir.dt.bfloat16)
        nc.gpsimd.dma_gather(
            out_ap=warm_out[:],
            in_ap=x_d[0:HALF, :],
            idxs_ap=warm_idx[:],
            num_idxs=P,
            num_idxs_reg=P,
            elem_size=C,
            single_packet=False,
        )

        x_lo = x_d[0:HALF, :]
        x_hi = x_d[HALF:NPAD, :]

        nbi_max = int((gather_nblk[:, 0] + gather_nblk[:, 1]).max())

        def issue_gather(ci):
            cblk0 = int(gather_blk0[ci, 0])
            nblk_c = int(gather_nblk[ci, 0] + gather_nblk[ci, 1])
            csblk0 = int(s_chunk0[ci])
            ns_c = int(s_chunk0[ci + 1]) - csblk0

            idx_sb = idx_p.tile([P, nbi_max * 8], mybir.dt.int16, tag="idx")
            nc.sync.dma_start(
                idx_sb[:, : nblk_c * 8], idx_d[:, cblk0 * 8 : (cblk0 + nblk_c) * 8]
            )

            s_sb = s_p.tile([P, nsc_max * P], mybir.dt.bfloat16, tag="s")
            nc.sync.dma_start(
                s_sb[:, : ns_c * P], s_d[:, csblk0 * P : (csblk0 + ns_c) * P]
            )

            gat_sb = gat_p.tile([P, nbc_max, P], mybir.dt.bfloat16, tag="gat")
            for h, src_ap in ((0, x_lo), (1, x_hi)):
                nblk_g = int(gather_nblk[ci, h])
                if nblk_g == 0:
                    continue
                nidx = nblk_g * P
                g0 = int(gather_blk0[ci, h]) - cblk0
                nc.gpsimd.dma_gather(
                    out_ap=gat_sb[:, g0 : g0 + nblk_g, :],
                    in_ap=src_ap,
                    idxs_ap=idx_sb[:, g0 * 8 : g0 * 8 + nidx // 16],
                    num_idxs=nidx,
                    num_idxs_reg=nidx,
                    elem_size=C,
                    single_packet=False,
                    queue_num=(ci + h) % 4,
                )
            return s_sb, gat_sb

        # chunk 0's gather goes first in program order so its DMA-sem lane
        # isn't queued behind the large meta loads
        pending = issue_gather(0)

        xself_sb = meta_p.tile([P, TPC * C], mybir.dt.bfloat16)
        nc.sync.dma_start(xself_sb[:], xself_d[:])
        wnT_sb = meta_p.tile([C, C], mybir.dt.float32)
        nc.sync.dma_start(wnT_sb[:], wnT_d[:])
        b_sb = meta_p.tile([1, C], mybir.dt.float32)
        nc.sync.dma_start(b_sb[:], b_d[:])
        ones_sb = meta_p.tile([1, C], mybir.dt.float32)
        nc.vector.memset(ones_sb[:], 1.0)
        eps_sb = meta_p.tile([P, 1], mybir.dt.float32)
        nc.vector.memset(eps_sb[:], EPS_IN)

        for ci, (t0, t1) in enumerate(SPANS):
            s_sb, gat_sb = pending
            if ci + 1 < NCHUNKS:
                pending = issue_gather(ci + 1)
            cblk0 = int(gather_blk0[ci, 0])
            csblk0 = int(s_chunk0[ci])

            for t in range(t0, t1):
                # (lhsT source, S block index within chunk) accumulation list
                mms = [("self", int(s_off_self[t]) - csblk0)]
                for h in range(2):
                    for j in range(int(nb[t, h])):
                        g = int(blk_off[t, h]) + j
                        mms.append((g - cblk0, int(s_off_gblk[g]) - csblk0))

                pt = ps_agg.tile([P, P], mybir.dt.float32)
                for j, (lsrc, sc_col) in enumerate(mms):
                    if lsrc == "self":
                        lhsT = xself_sb[:, t * C : (t + 1) * C]
                    else:
                        lhsT = gat_sb[:, lsrc, :]
                    nc.tensor.matmul(
                        pt[:],
                        lhsT=lhsT,
                        rhs=s_sb[:, sc_col * P : (sc_col + 1) * P],
                        start=(j == 0),
                        stop=(j == len(mms) - 1),
                    )

                agg_sb = agg_p.tile([P, P], mybir.dt.float32)
                nc.scalar.copy(agg_sb[:], pt[:])

                po = ps_out.tile([P, P], mybir.dt.float32)
                nc.tensor.matmul(po[:], lhsT=agg_sb[:], rhs=wnT_sb[:], start=True, stop=False)
                nc.tensor.matmul(po[:], lhsT=ones_sb[:], rhs=b_sb[:], start=False, stop=True)

                # InstanceNorm + LeakyReLU: stats on DVE (small), the full-tile
                # normalize+activate fused into one ACT Lrelu op.
                stats = small_p.tile([P, 6], mybir.dt.float32)
                nc.vector.bn_stats(out=stats[:], in_=po[:])
                mv = small_p.tile([P, 2], mybir.dt.float32)
                nc.vector.bn_aggr(out=mv[:], in_=stats[:])
                std = small_p.tile([P, 1], mybir.dt.float32)
                nc.scalar.activation(
                    out=std[:], in_=mv[:, 1:2],
                    func=mybir.ActivationFunctionType.Sqrt,
                    bias=eps_sb[:], scale=1.0,
                )
                rstd = small_p.tile([P, 1], mybir.dt.float32)
                nc.vector.reciprocal(out=rstd[:], in_=std[:])
                nbias = small_p.tile([P, 1], mybir.dt.float32)
                nc.vector.tensor_scalar(
                    out=nbias[:], in0=mv[:, 0:1], scalar1=rstd[:], scalar2=-1.0,
                    op0=mybir.AluOpType.mult, op1=mybir.AluOpType.mult,
                )
                y_sb = out_p.tile([P, P], mybir.dt.float32, tag="y")
                nc.scalar.activation(
                    out=y_sb[:], in_=po[:],
                    func=mybir.ActivationFunctionType.Identity,
                    bias=nbias[:], scale=rstd[:],
                )
                final = out_p.tile([P, P], mybir.dt.float32, tag="final")
                nc.vector.scalar_tensor_tensor(
                    out=final[:], in0=y_sb[:], scalar=0.2, in1=y_sb[:],
                    op0=mybir.AluOpType.mult, op1=mybir.AluOpType.max,
                )
                nc.sync.dma_start(out_d[t * P : (t + 1) * P, :], final[:])

    nc.compile()
    return nc


def _make_in_maps(x_pad, IDX, S_D, X_SELF, WnT, bvec):
    return [
        {
            "x": x_pad,
            "idx": np.ascontiguousarray(IDX[i]),
            "s": np.ascontiguousarray(S_D[i]),
            "xself": np.ascontiguousarray(X_SELF[i]),
            "wnT": WnT,
            "b": bvec,
        }
        for i in range(NCORES)
    ]


def kernel(x, edge_index, W, b, u):
    x_pad, IDX, S_D, X_SELF, WnT, bvec, meta = _preprocess(x, edge_index, W, b, u)
    nc = _build(meta)
    in_maps = _make_in_maps(x_pad, IDX, S_D, X_SELF, WnT, bvec)

    # The axon terminal can be transiently unavailable right after a prior
    # process's teardown; retry with backoff.
    import time

    last_err = None
    for attempt in range(6):
        try:
            res = run_bass_kernel_spmd(nc, in_maps, list(range(NCORES)))
            break
        except Exception as e:  # noqa: BLE001
            last_err = e
            time.sleep(45)
    else:
        raise last_err
    shards = [np.asarray(res.results[i]["out"]) for i in range(NCORES)]
    out = np.concatenate(shards, axis=0)[:N]
    return out.astype(np.float32)
